# revision 9
# baseline (speedup 1.0000x reference)
"""Trainium2 Bass kernel for nn_BlockV3 (dense transformer block).

Sharding: 8 cores = 2 (batch) x 4 (query-quarter). Each core holds the full
batch element for K/V and computes attention + MLP for its own 512 query
rows. Host-side prep reorders tokens per core (own 512 first) so the device
program is identical across cores (SPMD), and pre-transposes / pre-blocks /
bf16-casts the weights so the device kernel is fully feature-major with zero
on-chip transposes.

Folding done on host (exact fp32 algebra):
  - LN gains/biases fold into the following linear: W' = W*g, b' = W@b_ln + b
  - V-projection bias folds through attention (rows of att sum to 1) into the
    out-projection bias: bp'' = bp + Wp@bv'
  - the padding/cond mask becomes an additive bias fused into the exp on the
    scores; the softmax denominator is recovered via an extra ones-column in
    the att@V matmul and divided out with a per-head broadcast matmul.
"""

import sys
import numpy as np

sys.path.insert(0, "/opt/trn_rl_repo")

B = 2
T = 2048
C = 768
H = 12
Dh = 64
F = 3072
P = 128
NCH = C // P          # 6 feature chunks
NFT = F // P          # 24 mlp chunks
NKT = T // P          # 16 key tiles
TQ = 512              # own query rows per core
NQ4 = T // TQ         # 4 t-quarters
N_CORES = 8
EPS = 1e-5

_CACHE = {}


def _build_nc():
    import concourse.bass as bass
    from concourse import bacc, mybir
    import concourse.tile as tile

    f32 = mybir.dt.float32
    bf16 = mybir.dt.bfloat16

    nc = bacc.Bacc()
    eps_t = nc.alloc_sbuf_tensor("const-eps", [128, 1], f32)
    nc.gpsimd.memset(eps_t.ap(), EPS)
    nc.const_aps.aps[(f32, EPS)] = eps_t.ap()

    d = {}
    d["xT"] = nc.declare_dram_parameter("xT", [C, T], bf16, isOutput=False)
    d["xTown"] = nc.declare_dram_parameter("xTown", [C, TQ], f32, isOutput=False)
    d["mbias"] = nc.declare_dram_parameter("mbias", [T], f32, isOutput=False)
    d["wqB"] = nc.declare_dram_parameter("wqB", [NCH, P, NCH, P], bf16, isOutput=False)
    d["wkB"] = nc.declare_dram_parameter("wkB", [NCH, P, NCH, P], bf16, isOutput=False)
    d["wvR"] = nc.declare_dram_parameter("wvR", [NCH, P, C], bf16, isOutput=False)
    d["wpB"] = nc.declare_dram_parameter("wpB", [NCH, P, NCH, P], bf16, isOutput=False)
    d["w1B"] = nc.declare_dram_parameter("w1B", [NFT, P, NCH, P], bf16, isOutput=False)
    d["w2B"] = nc.declare_dram_parameter("w2B", [NCH, P, NFT, P], bf16, isOutput=False)
    d["bqR"] = nc.declare_dram_parameter("bqR", [P, NCH], f32, isOutput=False)
    d["bkR"] = nc.declare_dram_parameter("bkR", [P, NCH], f32, isOutput=False)
    d["boR"] = nc.declare_dram_parameter("boR", [P, NCH], f32, isOutput=False)
    d["b1R"] = nc.declare_dram_parameter("b1R", [P, NFT], f32, isOutput=False)
    d["b2R"] = nc.declare_dram_parameter("b2R", [P, NCH], f32, isOutput=False)
    d["sel"] = nc.declare_dram_parameter("sel", [H, C], bf16, isOutput=False)
    d["outT"] = nc.declare_dram_parameter("outT", [C, TQ], f32, isOutput=True)

    with tile.TileContext(nc) as tc:
        _emit(tc, nc, mybir, bass, tile, d)
    nc.finalize()
    return nc


def _emit(tc, nc, mybir, bass, tile, g):
    from contextlib import ExitStack

    f32 = mybir.dt.float32
    bf16 = mybir.dt.bfloat16
    AF = mybir.ActivationFunctionType
    OP = mybir.AluOpType
    ts = bass.ts

    xT, xTown, mbias = g["xT"], g["xTown"], g["mbias"]
    wqB, wkB, wvR, wpB, w1B, w2B = (g["wqB"], g["wkB"], g["wvR"], g["wpB"],
                                    g["w1B"], g["w2B"])
    bqR, bkR, boR, b1R, b2R, selD, outT = (
        g["bqR"], g["bkR"], g["boR"], g["b1R"], g["b2R"], g["sel"], g["outT"])

    ctx = ExitStack()
    with ctx:
        psum = ctx.enter_context(tc.tile_pool(name="psum", bufs=4, space="PSUM"))

        def pt(name):
            # uniform 2-bank psum tile [P, 2, TQ]; slice what you need
            return psum.tile([P, 2, TQ], f32, tag="mm", bufs=4, name=name)
        sb = ctx.enter_context(tc.tile_pool(name="sb", bufs=1))

        # 4KB/partition slots shared across phases: x chunks + squares +
        # u1 (phase 1-2) then e^T tiles (phase 3)
        def big(name):
            return sb.tile([P, T], bf16, tag="big", bufs=10, name=name)

        class PoolShim:
            def __init__(self, tag, bufs):
                self.tag, self.bufs = tag, bufs

            def tile(self, shape, dtype, tag=None, name=None):
                return sb.tile(shape, dtype, tag=self.tag, bufs=self.bufs,
                               name=name or f"{self.tag}t")

        xs_pool = PoolShim("big", 10)
        sq_pool = PoolShim("big", 10)
        u1_pool = PoolShim("big", 10)
        et_pool = PoolShim("big", 10)
        kt_pool = PoolShim("kt", NCH)
        vp_pool = PoolShim("vp", NKT)
        mid_pool = PoolShim("ab", 8)          # a4/b4 + a2b/b2b
        qt_pool = PoolShim("qu", NCH)         # q^T then u2
        ys_pool = PoolShim("ysgt", NFT)       # ystack then gelu tiles
        rb_pool = PoolShim("rb", 2)
        yc_pool = PoolShim("yc", 2)
        gt_pool = ys_pool
        x2_pool = PoolShim("x2", NCH)
        f2_pool = PoolShim("f2", 2)
        w_pool = PoolShim("w15", 9)           # weight blocks + wv rows
        w2_pool = PoolShim("w2st", 2)
        wv_pool = PoolShim("w15", 9)
        row_pool = PoolShim("rows", 4)
        small = PoolShim("small", 1)

        # ---- constants / small loads ----
        mb = sb.tile([P, NKT], f32, tag="mb", bufs=1, name="mb")
        nc.sync.dma_start(mb, mbias[:].rearrange("(c p) -> p c", p=P))
        bq_s = sb.tile([P, NCH], f32, tag="bq", bufs=1, name="bq_s")
        nc.sync.dma_start(bq_s, bqR[:, :])
        bk_s = sb.tile([P, NCH], f32, tag="bk", bufs=1, name="bk_s")
        nc.sync.dma_start(bk_s, bkR[:, :])
        bo_s = sb.tile([P, NCH], f32, tag="bo", bufs=1, name="bo_s")
        nc.sync.dma_start(bo_s, boR[:, :])
        b1_s = sb.tile([P, NFT], f32, tag="b1", bufs=1, name="b1_s")
        nc.sync.dma_start(b1_s, b1R[:, :])
        b2_s = sb.tile([P, NCH], f32, tag="b2", bufs=1, name="b2_s")
        nc.sync.dma_start(b2_s, b2R[:, :])
        sel_s = sb.tile([H, C], bf16, tag="sel", bufs=1, name="sel_s")
        nc.sync.dma_start(sel_s, selD[:, :])
        ones_b = sb.tile([P, 1], bf16, tag="ones_b", bufs=1, name="ones_b")
        nc.vector.memset(ones_b, 1.0)
        ones_f = sb.tile([P, 1], f32, tag="ones_f", bufs=1, name="ones_f")
        nc.vector.memset(ones_f, 1.0)
        ones_rf = sb.tile([1, P], f32, tag="ones_rf", bufs=1, name="ones_rf")
        nc.vector.memset(ones_rf, 1.0)

        def ln_rows(s1p_q, s2p_q):
            """psum sums [1,512] -> (a_row, b_row) [1,512] f32 tiles."""
            mu = row_pool.tile([1, TQ], f32, tag="row")
            nc.vector.tensor_scalar_mul(mu, s1p_q, 1.0 / C)
            var = row_pool.tile([1, TQ], f32, tag="row")
            nc.vector.tensor_scalar_mul(var, s2p_q, 1.0 / C)
            musq = row_pool.tile([1, TQ], f32, tag="row")
            nc.vector.tensor_tensor(musq, mu, mu, OP.mult)
            nc.vector.tensor_tensor(var, var, musq, OP.subtract)
            # rstd = exp(-0.5 * ln(var + eps)); ln/exp share one ACT table set
            a_r = row_pool.tile([1, TQ], f32, tag="row")
            nc.scalar.activation(a_r, var, AF.Ln, bias=EPS, scale=1.0)
            nc.scalar.activation(a_r, a_r, AF.Exp, bias=0.0, scale=-0.5)
            b_r = row_pool.tile([1, TQ], f32, tag="row")
            nc.vector.tensor_tensor(b_r, mu, a_r, OP.mult)
            nc.vector.tensor_scalar_mul(b_r, b_r, -1.0)
            return a_r, b_r

        def bcast128(row, name="bc"):
            """[1,512] f32 row -> [128,512] bf16 tile via K=1 matmul."""
            pp = pt(name)[:, 0, :]
            nc.tensor.matmul(pp, ones_rf, row, start=True, stop=True)
            out = mid_pool.tile([P, TQ], bf16, tag="ab", name=name + "s")
            nc.vector.tensor_copy(out, pp)
            return out

        # ================= Phase 1: LN1 statistics over full T =================
        s1t = [pt(f"s1t{i}") for i in range(2)]
        s2t = [pt(f"s2t{i}") for i in range(2)]
        s1p = [s1t[q // 2][0:1, q % 2, :] for q in range(NQ4)]
        s2p = [s2t[q // 2][0:1, q % 2, :] for q in range(NQ4)]
        for c in range(NCH):
            xt = xs_pool.tile([P, T], bf16, tag="xt")
            nc.sync.dma_start(xt, xT[c * P:(c + 1) * P, :])
            xsq = sq_pool.tile([P, T], bf16, tag="xsq")
            nc.vector.tensor_tensor(xsq, xt, xt, OP.mult)
            for q in range(NQ4):
                nc.tensor.matmul(s1p[q], ones_b, xt[:, ts(q, TQ)],
                                 start=(c == 0), stop=(c == NCH - 1))
                nc.tensor.matmul(s2p[q], ones_b, xsq[:, ts(q, TQ)],
                                 start=(c == 0), stop=(c == NCH - 1))
        a4 = [None] * NQ4
        b4 = [None] * NQ4
        for pair in range(2):
            rows = [ln_rows(s1p[2 * pair + j], s2p[2 * pair + j])
                    for j in range(2)]
            for j in range(2):
                q = 2 * pair + j
                a4[q] = bcast128(rows[j][0], name=f"a4_{q}")
                b4[q] = bcast128(rows[j][1], name=f"b4_{q}")

        # ================= Phase 2: LN1 apply + QKV projections ================
        u1 = []
        for c in range(NCH):
            xt = xs_pool.tile([P, T], bf16, tag="xt")
            nc.sync.dma_start(xt, xT[c * P:(c + 1) * P, :])
            u = u1_pool.tile([P, T], bf16, tag="u1")
            for q in range(NQ4):
                nc.vector.tensor_tensor(u[:, ts(q, TQ)], xt[:, ts(q, TQ)], a4[q],
                                        OP.mult)
                nc.vector.tensor_tensor(u[:, ts(q, TQ)], u[:, ts(q, TQ)], b4[q],
                                        OP.add)
            u1.append(u)

        # Q projection: feature-major q^T [C, TQ] (own rows only)
        qt = []
        for ot in range(NCH):
            wq = w_pool.tile([P, NCH, P], bf16, tag="wblk")
            nc.sync.dma_start(wq, wqB[ot])
            qp = pt(f"qp{ot}")[:, 0, :]
            for kc in range(NCH):
                nc.tensor.matmul(qp, wq[:, kc, :], u1[kc][:, 0:TQ],
                                 start=(kc == 0), stop=(kc == NCH - 1))
            qs = qt_pool.tile([P, TQ], bf16, name="qt")
            nc.vector.tensor_scalar_add(qs, qp, bq_s[:, ot:ot + 1])
            qt.append(qs)

        # K projection: feature-major k^T [C, T] (full batch element)
        kt = []
        for ot in range(NCH):
            wk = w_pool.tile([P, NCH, P], bf16, tag="wblk")
            nc.sync.dma_start(wk, wkB[ot])
            ks = kt_pool.tile([P, T], bf16, tag="kt")
            for g in range(2):
                kp = pt(f"kp{ot}_{g}")
                for kc in range(NCH):
                    for j in range(2):
                        nc.tensor.matmul(kp[:, j, :], wk[:, kc, :],
                                         u1[kc][:, ts(2 * g + j, TQ)],
                                         start=(kc == 0), stop=(kc == NCH - 1))
                out3 = ks[:, ts(g, 2 * TQ)].rearrange("p (a b) -> p a b", b=TQ)
                nc.vector.tensor_scalar_add(out3, kp, bk_s[:, ot:ot + 1])
            kt.append(ks)

        # V projection: token-major v [T, C], stored with 12 interleaved
        # ones-columns: head h occupies cols h*65..h*65+63, col h*65+64 = 1.0
        # (the ones column turns att@V into att@V plus the softmax denominator).
        wv = []
        for kc in range(NCH):
            w = wv_pool.tile([P, C], bf16, tag="wv")
            nc.sync.dma_start(w, wvR[kc])
            wv.append(w)
        # v rows of masked-out keys are zeroed and the per-head 65th column
        # holds the 0/1 mask itself, so att@v' yields both the masked
        # numerator and the masked softmax denominator with unmasked exp.
        vt = []
        for tk in range(NKT):
            vp_ = pt(f"vp{tk}")
            va = vp_[:, 0, :]
            vb = vp_[:, 1, 0:256]
            for kc in range(NCH):
                lhs = u1[kc][:, ts(tk, P)]
                nc.tensor.matmul(va, lhs, wv[kc][:, 0:512],
                                 start=(kc == 0), stop=(kc == NCH - 1))
                nc.tensor.matmul(vb, lhs, wv[kc][:, 512:768],
                                 start=(kc == 0), stop=(kc == NCH - 1))
            v = vp_pool.tile([P, H, 65], bf16, tag="vp")
            va3 = va.rearrange("p (h d) -> p h d", d=64)
            vb3 = vb.rearrange("p (h d) -> p h d", d=64)
            mcol = mb[:, tk:tk + 1]
            nc.vector.tensor_scalar_mul(v[:, 0:8, 0:64], va3, mcol)
            nc.vector.tensor_scalar_mul(v[:, 8:12, 0:64], vb3, mcol)
            nc.vector.tensor_copy(v[:, :, 64:65], mcol.to_broadcast((P, H, 1)))
            vt.append(v)

        # ================= Phase 3: attention =================
        den = sb.tile([H, TQ], bf16, tag="den", bufs=1, name="den")
        ystack = [ys_pool.tile([P, TQ], bf16, name=f"ystack{i}") for i in range(NCH)]
        for hp in range(NCH):
            ets2 = []
            for h2 in range(2):
                ets2.append([et_pool.tile([P, 4 * TQ], bf16, tag="et",
                                          name=f"et{hp}_{h2}_{i}")
                             for i in range(4)])
            for tg in range(NKT // 2):
                sps = [pt(f"sp{hp}_{tg}_{h2}") for h2 in range(2)]
                for j in range(2):
                    tk = 2 * tg + j
                    # two K=64 matmuls on disjoint row groups run
                    # concurrently in the PE array (tile_position rows)
                    for h2 in range(2):
                        rows = slice(64 * h2, 64 * h2 + 64)
                        nc.tensor.matmul(sps[h2][:, j, :],
                                         kt[hp][rows, ts(tk, P)],
                                         qt[hp][rows, :],
                                         start=True, stop=True)
                for h2 in range(2):
                    out3 = ets2[h2][tg // 2][:, ts(tg % 2, 2 * TQ)].rearrange(
                        "p (a b) -> p a b", b=TQ)
                    nc.scalar.activation(out3, sps[h2], AF.Exp,
                                         bias=0.0, scale=0.125)
            for h2 in range(2):
                h = 2 * hp + h2
                rows = slice(64 * h2, 64 * h2 + 64)
                ets = ets2[h2]
                ya = pt(f"ya{h}")[0:65, 0, :]
                for tk in range(NKT):
                    nc.tensor.matmul(ya, vt[tk][:, h, :],
                                     ets[tk // 4][:, ts(tk % 4, TQ)],
                                     start=(tk == 0), stop=(tk == NKT - 1))
                yc = yc_pool.tile([65, TQ], bf16, tag="yc")
                nc.vector.tensor_copy(yc, ya)
                # cross-partition moves go through SBUF->SBUF DMA
                nc.sync.dma_start(ystack[hp][rows, :], yc[0:64, :])
                nc.sync.dma_start(den[h:h + 1, :], yc[64:65, :])
        # r = 1/den via exp(-ln(den)); broadcast to the 64 rows of each head
        # with a one-hot [12,128] matmul, then scale y.
        rr = sb.tile([H, TQ], bf16, tag="rr", bufs=1, name="rr")
        nc.scalar.activation(rr, den, AF.Ln, bias=0.0, scale=1.0)
        nc.scalar.activation(rr, rr, AF.Exp, bias=0.0, scale=-1.0)
        for hp in range(NCH):
            rp = pt(f"rp{hp}")[:, 0, :]
            nc.tensor.matmul(rp, sel_s[:, ts(hp, P)], rr, start=True, stop=True)
            rb = rb_pool.tile([P, TQ], bf16, tag="rb")
            nc.vector.tensor_copy(rb, rp)
            nc.vector.tensor_tensor(ystack[hp], ystack[hp], rb, OP.mult)

        # ================= Phase 4: out-projection + residual =================
        x2t = []
        for ot in range(NCH):
            wp = w_pool.tile([P, NCH, P], bf16, tag="wblk")
            nc.sync.dma_start(wp, wpB[ot])
            xp = pt(f"xp{ot}")[:, 0, :]
            for kc in range(NCH):
                nc.tensor.matmul(xp, wp[:, kc, :], ystack[kc],
                                 start=(kc == 0), stop=(kc == NCH - 1))
            x2 = x2_pool.tile([P, TQ], f32, tag="x2t")
            nc.vector.tensor_scalar_add(x2, xp, bo_s[:, ot:ot + 1])
            xo = f2_pool.tile([P, TQ], f32, tag="xtown")
            nc.sync.dma_start(xo, xTown[ot * P:(ot + 1) * P, :])
            nc.vector.tensor_tensor(x2, x2, xo, OP.add)
            x2t.append(x2)

        # ================= Phase 5: LN2 (own rows) =================
        sp2t = pt("sp2t")
        s1p2 = sp2t[0:1, 0, :]
        s2p2 = sp2t[0:1, 1, :]
        for c in range(NCH):
            xsq2 = f2_pool.tile([P, TQ], f32, tag="xsq2")
            nc.vector.tensor_tensor(xsq2, x2t[c], x2t[c], OP.mult)
            nc.tensor.matmul(s1p2, ones_f, x2t[c], start=(c == 0),
                             stop=(c == NCH - 1))
            nc.tensor.matmul(s2p2, ones_f, xsq2, start=(c == 0),
                             stop=(c == NCH - 1))
        a2_r, b2_r = ln_rows(s1p2, s2p2)
        a2b = bcast128(a2_r, name="a2b")
        b2b = bcast128(b2_r, name="b2b")
        u2 = []
        for c in range(NCH):
            u = qt_pool.tile([P, TQ], bf16, name="u2")
            nc.vector.tensor_tensor(u, x2t[c], a2b, OP.mult)
            nc.vector.tensor_tensor(u, u, b2b, OP.add)
            u2.append(u)

        # ================= Phase 6: MLP =================
        gt = []
        for mt in range(NFT):
            w1 = w_pool.tile([P, NCH, P], bf16, tag="wblk")
            nc.sync.dma_start(w1, w1B[mt])
            mp = pt(f"mp{mt}")[:, 0, :]
            for kc in range(NCH):
                nc.tensor.matmul(mp, w1[:, kc, :], u2[kc],
                                 start=(kc == 0), stop=(kc == NCH - 1))
            gs = gt_pool.tile([P, TQ], bf16, tag="gt")
            nc.scalar.activation(gs, mp, AF.Gelu, bias=b1_s[:, mt:mt + 1],
                                 scale=1.0)
            gt.append(gs)
        for ot in range(NCH):
            w2a = w2_pool.tile([P, NFT // 2, P], bf16, tag="w2blk")
            nc.sync.dma_start(w2a, w2B[ot, :, 0:NFT // 2, :])
            w2b = w2_pool.tile([P, NFT // 2, P], bf16, tag="w2blk")
            nc.sync.dma_start(w2b, w2B[ot, :, NFT // 2:NFT, :])
            op_ = pt(f"op{ot}")[:, 0, :]
            for kc in range(NFT):
                wsl = w2a[:, kc, :] if kc < NFT // 2 else w2b[:, kc - NFT // 2, :]
                nc.tensor.matmul(op_, wsl, gt[kc],
                                 start=(kc == 0), stop=(kc == NFT - 1))
            ot_s = f2_pool.tile([P, TQ], f32, tag="outt")
            nc.vector.tensor_scalar_add(ot_s, op_, b2_s[:, ot:ot + 1])
            nc.vector.tensor_tensor(ot_s, ot_s, x2t[ot], OP.add)
            nc.sync.dma_start(outT[ot * P:(ot + 1) * P, :], ot_s)


def _get_nc():
    if "nc" not in _CACHE:
        _CACHE["nc"] = _build_nc()
    return _CACHE["nc"]


def _host_prep(inputs):
    import ml_dtypes
    bf = ml_dtypes.bfloat16

    x = np.asarray(inputs["x"], np.float32)
    cond_len = int(np.asarray(inputs["cond_len"]))
    pm = np.asarray(inputs["padding_mask"])
    g1 = np.asarray(inputs["g1"], np.float32)
    bln1 = np.asarray(inputs["bln1"], np.float32)
    g2 = np.asarray(inputs["g2"], np.float32)
    bln2 = np.asarray(inputs["bln2"], np.float32)
    Wq = np.asarray(inputs["Wq"], np.float32)
    Wk = np.asarray(inputs["Wk"], np.float32)
    Wv = np.asarray(inputs["Wv"], np.float32)
    Wp = np.asarray(inputs["Wp"], np.float32)
    W1 = np.asarray(inputs["W1"], np.float32)
    W2 = np.asarray(inputs["W2"], np.float32)
    bq = np.asarray(inputs["bq"], np.float32)
    bk = np.asarray(inputs["bk"], np.float32)
    bv = np.asarray(inputs["bv"], np.float32)
    bp = np.asarray(inputs["bp"], np.float32)
    b1 = np.asarray(inputs["b1"], np.float32)
    b2 = np.asarray(inputs["b2"], np.float32)

    Wq_ = Wq * g1[None, :]
    Wk_ = Wk * g1[None, :]
    Wv_ = Wv * g1[None, :]
    bq_ = Wq @ bln1 + bq
    bk_ = Wk @ bln1 + bk
    bv_ = Wv @ bln1 + bv
    bp_ = bp + Wp @ bv_
    W1_ = W1 * g2[None, :]
    b1_ = W1 @ bln2 + b1

    def blk(WT):
        # WT [K, M] -> [M/128, 128(kp), K/128, 128(m)]
        Kd, Md = WT.shape
        return np.ascontiguousarray(
            WT.reshape(Kd // P, P, Md // P, P).transpose(2, 1, 0, 3)).astype(bf)

    def bre(b):
        return np.ascontiguousarray(b.reshape(-1, P).T).astype(np.float32)

    sel = np.zeros((H, C), bf)
    for h in range(H):
        sel[h, h * Dh:(h + 1) * Dh] = 1.0

    n_b = T - pm.sum(axis=1)
    cols = np.arange(T)
    allowed = (cols[None, :] >= cond_len) | (cols[None, :] < np.asarray(n_b)[:, None])
    M = allowed.astype(np.float32)

    shared = dict(
        wqB=blk(Wq_.T), wkB=blk(Wk_.T),
        wvR=np.ascontiguousarray(Wv_.T.reshape(NCH, P, C)).astype(bf),
        wpB=blk(Wp.T), w1B=blk(W1_.T), w2B=blk(W2.T),
        bqR=bre(bq_), bkR=bre(bk_), boR=bre(bp_), b1R=bre(b1_), b2R=bre(b2),
        sel=sel)

    in_maps = []
    perms = []
    for core in range(N_CORES):
        b = core // 4
        qi = core % 4
        own = np.arange(qi * TQ, (qi + 1) * TQ)
        rest = np.concatenate([np.arange(0, qi * TQ), np.arange((qi + 1) * TQ, T)])
        perm = np.concatenate([own, rest])
        perms.append((b, qi))
        xb = x[b]
        m = dict(shared)
        m.update(
            xT=np.ascontiguousarray(xb[perm].T).astype(bf),
            xTown=np.ascontiguousarray(xb[own].T).astype(np.float32),
            mbias=np.ascontiguousarray(M[b][perm]))
        in_maps.append(m)
    return in_maps, perms


def kernel(**inputs):
    from concourse.bass_utils import run_bass_kernel_spmd

    nc = _get_nc()
    in_maps, perms = _host_prep(inputs)
    res = run_bass_kernel_spmd(nc, in_maps, list(range(N_CORES)),
                               **_CACHE.get("run_kwargs", {}))
    _CACHE["last_results"] = res
    x = np.asarray(inputs["x"])
    out = np.zeros((B, T, C), np.float32)
    for core in range(N_CORES):
        b, qi = perms[core]
        out[b, qi * TQ:(qi + 1) * TQ, :] = res.results[core]["outT"].T
    return out.astype(x.dtype)
